# revision 5
# baseline (speedup 1.0000x reference)
"""Multi-head causal attention with RoPE on 8 Trainium2 NeuronCores.

Problem: x[2, 2048, 1024], 16 heads, d_k=64, RoPE(theta=1e4), causal,
weights W{q,k,v,o}[1024, 1024] stored [d_out, d_in].

Sharding: 2 batches x 4 head-groups -> 8 cores. Core c handles batch
c//4, heads 4*(c%4) .. 4*(c%4)+4. Each core computes its 4 heads'
attention plus the partial o_proj for its head columns; the host sums
the 4 partials per batch (the "all-reduce after o_proj").

Device kernel layout choices (per core):
- Q/K are produced in a permuted [e', s] layout, e' = parity*128 +
  h*32 + j (parity = RoPE pair element, j = rotation freq index), so
  RoPE is 6 full-width DVE ops per 512-seq chunk and the score matmuls
  contract head h over partition rows [32h, 32h+32) of both parity
  tiles (row-packed via tile_position, 4 heads concurrently).
- Scores are computed key-major (scoresT [k, q]) so the attn@V matmul
  needs no transpose and the softmax denominator rides the V matmul as
  an appended ones-column (out row 64).
- The causal mask on diagonal k-tiles is added in PSUM by one extra
  accumulating matmul: identity.T @ mask_pattern (patterns host-built).
- All matmul operands are float32r (TF32-class, full PE rate at N>=256).
"""

import sys

if "/opt/trn_rl_repo" not in sys.path:
    sys.path.insert(0, "/opt/trn_rl_repo")

import numpy as np

import concourse.bass as bass
import concourse.mybir as mybir
import concourse.tile as tile
from concourse import bacc, library_config
from concourse.bass_utils import run_bass_kernel_spmd

F32 = mybir.dt.float32
F32R = mybir.dt.float32r
EXP = mybir.ActivationFunctionType.Exp

B = 2
S = 2048
D = 1024
H = 16
DK = 64
HC = 4          # heads per core
E = HC * DK     # 256 d_out columns per core
THETA = 10000.0
SC = 512        # seq chunk (psum free dim)
NSC = S // SC   # 4
NST = S // 128  # 16 s-tiles
NEG = -1.0e30

_COMPILED = None


def _build():
    nc = bacc.Bacc("TRN2", target_bir_lowering=False, debug=False, num_devices=8)

    xT = nc.dram_tensor("xT", [D, S], F32, kind="ExternalInput")
    wqT = nc.dram_tensor("wqT", [D, E], F32, kind="ExternalInput")
    wkT = nc.dram_tensor("wkT", [D, E], F32, kind="ExternalInput")
    wvT = nc.dram_tensor("wvT", [D, E], F32, kind="ExternalInput")
    woT = nc.dram_tensor("woT", [E, D], F32, kind="ExternalInput")
    cosT = nc.dram_tensor("cosT", [128, S], F32, kind="ExternalInput")
    sinT = nc.dram_tensor("sinT", [128, S], F32, kind="ExternalInput")
    masks = nc.dram_tensor("masks", [4, 128, SC], F32, kind="ExternalInput")
    eye = nc.dram_tensor("eye", [128, 128], F32, kind="ExternalInput")
    ones = nc.dram_tensor("ones", [128, NST, HC], F32, kind="ExternalInput")
    out_d = nc.dram_tensor("out", [S, D], F32, kind="ExternalOutput")

    with tile.TileContext(nc) as tc:
        with (
            tc.tile_pool(name="const", bufs=1) as const,
            tc.tile_pool(name="persist", bufs=1) as persist,
            tc.tile_pool(name="xp", bufs=2) as xp,
            tc.tile_pool(name="ropet", bufs=2) as ropet,
            tc.tile_pool(name="expool", bufs=2) as expool,
            tc.tile_pool(name="rpool", bufs=2) as rpool,
            tc.tile_pool(name="opool", bufs=2) as opool,
            tc.tile_pool(name="cspool", bufs=2) as cspool,
        ):
            nc.gpsimd.load_library(library_config.attn)

            # ---- constant loads -------------------------------------
            wq_sb = const.tile([128, 8, E], F32R)
            wk_sb = const.tile([128, 8, E], F32R)
            wv_sb = const.tile([128, 8, E], F32R)
            nc.sync.dma_start(
                wq_sb[:], wqT[:].rearrange("(c p) e -> p c e", p=128).bitcast(F32R))
            nc.sync.dma_start(
                wk_sb[:], wkT[:].rearrange("(c p) e -> p c e", p=128).bitcast(F32R))
            nc.sync.dma_start(
                wv_sb[:], wvT[:].rearrange("(c p) e -> p c e", p=128).bitcast(F32R))
            wo_sb = const.tile([128, 2, D], F32R)
            nc.sync.dma_start(
                wo_sb[:], woT[:].rearrange("(c p) d -> p c d", p=128).bitcast(F32R))
            mask_sb = const.tile([128, 4, SC], F32R)
            nc.sync.dma_start(
                mask_sb[:], masks[:].rearrange("m k q -> k m q").bitcast(F32R))
            eye_sb = const.tile([128, 128], F32R)
            nc.sync.dma_start(eye_sb[:], eye[:].bitcast(F32R))

            # ---- persistent activations -----------------------------
            q0_sb = persist.tile([128, S], F32R)   # parity-0 rotated Q
            q1_sb = persist.tile([128, S], F32R)
            k0_sb = persist.tile([128, S], F32R)
            k1_sb = persist.tile([128, S], F32R)
            v_sb = persist.tile([128, NST, HC * 65], F32R)  # [k, s_tile, h*65+dk | ones]
            ao_sb = persist.tile([128, 2, S], F32R)         # o_proj lhsT, pair-major

            v3 = v_sb[:].rearrange("p t (h c) -> p t h c", c=65)
            nc.sync.dma_start(
                v3[:, :, :, 64:65],
                ones[:].rearrange("p t h -> p t h 1" if False else "p t (h o) -> p t h o", o=1)
                .bitcast(F32R))

            # ---- stage 1: QKV projections + RoPE + V layout ---------
            with tc.tile_pool(name="ps1", bufs=1, space="PSUM") as ps1:
                for c in range(NSC):
                    sl = slice(SC * c, SC * (c + 1))
                    x_sb = xp.tile([128, 8, SC], F32R, name=f"x_{c}", tag="x")
                    nc.sync.dma_start(
                        x_sb[:],
                        xT[:].rearrange("(dc p) s -> p dc s", p=128)[:, :, sl]
                        .bitcast(F32R))

                    pq = [ps1.tile([128, SC], F32, name=f"pq{t}_{c}", tag=f"pq{t}")
                          for t in range(2)]
                    pk = [ps1.tile([128, SC], F32, name=f"pk{t}_{c}", tag=f"pk{t}")
                          for t in range(2)]
                    for t in range(2):
                        es = slice(128 * t, 128 * (t + 1))
                        for dc in range(8):
                            nc.tensor.matmul(
                                pq[t][:], wq_sb[:, dc, es], x_sb[:, dc, :],
                                start=(dc == 0), stop=(dc == 7))
                        for dc in range(8):
                            nc.tensor.matmul(
                                pk[t][:], wk_sb[:, dc, es], x_sb[:, dc, :],
                                start=(dc == 0), stop=(dc == 7))
                    pv = [ps1.tile([128, 2, 256], F32, name=f"pv{t}_{c}", tag=f"pv{t}")
                          for t in range(2)]
                    for st in range(4):
                        ssl = slice(128 * st, 128 * (st + 1))
                        for dc in range(8):
                            nc.tensor.matmul(
                                pv[st // 2][:, st % 2, :],
                                x_sb[:, dc, ssl], wv_sb[:, dc, :],
                                start=(dc == 0), stop=(dc == 7))

                    # RoPE: x1' = x1 c - x2 s ; x2' = x1 s + x2 c
                    cs_sb = cspool.tile([128, SC], F32, name=f"cos_{c}", tag="cos")
                    sn_sb = cspool.tile([128, SC], F32, name=f"sin_{c}", tag="sin")
                    nc.sync.dma_start(cs_sb[:], cosT[:, sl])
                    nc.sync.dma_start(sn_sb[:], sinT[:, sl])
                    C = cs_sb[:]
                    Sn = sn_sb[:]
                    for name, p0, p1, d0, d1 in (
                        ("q", pq[0], pq[1], q0_sb, q1_sb),
                        ("k", pk[0], pk[1], k0_sb, k1_sb),
                    ):
                        t0 = ropet.tile([128, SC], F32, name=f"t0{name}{c}", tag="ta")
                        t1 = ropet.tile([128, SC], F32, name=f"t1{name}{c}", tag="tb")
                        t2 = ropet.tile([128, SC], F32, name=f"t2{name}{c}", tag="ta")
                        t3 = ropet.tile([128, SC], F32, name=f"t3{name}{c}", tag="tb")
                        nc.vector.tensor_mul(t0[:], p0[:], C)
                        nc.vector.tensor_mul(t1[:], p1[:], Sn)
                        nc.vector.tensor_sub(d0[:, sl], t0[:], t1[:])
                        nc.vector.tensor_mul(t2[:], p0[:], Sn)
                        nc.vector.tensor_mul(t3[:], p1[:], C)
                        nc.vector.tensor_add(d1[:, sl], t2[:], t3[:])

                    # V into [k, h*65+dk] layout (ones col preset above)
                    for st in range(4):
                        nc.scalar.copy(
                            v3[:, 4 * c + st, :, 0:64],
                            pv[st // 2][:, st % 2, :]
                            .rearrange("p (h c) -> p h c", c=64))

            # ---- stage 2: attention ---------------------------------
            with tc.tile_pool(name="ps2", bufs=1, space="PSUM") as ps2:
                for qc in range(NSC):
                    qsl = slice(SC * qc, SC * (qc + 1))
                    av = [ps2.tile([128, SC], F32, name=f"av{h}_{qc}", tag=f"av{h}")
                          for h in range(HC)]
                    nkt = 4 * qc + 4
                    for kt in range(nkt):
                        ksl = slice(128 * kt, 128 * (kt + 1))
                        diag = kt >= 4 * qc
                        w = 128 * (kt - 4 * qc) if diag else 0
                        m = kt - 4 * qc
                        for h in range(HC):
                            hp = slice(32 * h, 32 * (h + 1))
                            tp = (96, 0) if h == 3 else None
                            sc_ps = ps2.tile([128, SC], F32,
                                             name=f"sc{h}_{qc}_{kt}", tag=f"sc{h}")
                            nc.tensor.matmul(
                                sc_ps[:, w:SC], k0_sb[hp, ksl],
                                q0_sb[hp, qsl][:, w:SC],
                                start=True, stop=False, tile_position=tp)
                            nc.tensor.matmul(
                                sc_ps[:, w:SC], k1_sb[hp, ksl],
                                q1_sb[hp, qsl][:, w:SC],
                                start=False, stop=not diag, tile_position=tp)
                            if diag:
                                nc.tensor.matmul(
                                    sc_ps[:, w:SC], eye_sb[:],
                                    mask_sb[:, m, w:SC],
                                    start=False, stop=True)
                            ex = expool.tile([128, SC], F32R,
                                             name=f"ex{h}_{qc}_{kt}", tag=f"ex{h}")
                            nc.scalar.activation(ex[:, w:SC], sc_ps[:, w:SC], EXP)
                            nc.tensor.matmul(
                                av[h][0:65, w:SC], v_sb[:, kt, 65 * h:65 * h + 65],
                                ex[:, w:SC],
                                start=(kt == 0), stop=(kt == nkt - 1))

                    for h in range(HC):
                        rb = rpool.tile([64, SC], F32, name=f"rb{h}_{qc}", tag="rb")
                        nc.vector.reciprocal(rb[0:1, :], av[h][64:65, :])
                        nc.gpsimd.partition_broadcast(rb[:], rb[0:1, :])
                        u, pr = h % 2, h // 2
                        nc.vector.tensor_mul(
                            ao_sb[64 * u:64 * u + 64, pr, qsl],
                            av[h][0:64, :], rb[:])

            # ---- stage 3: o_proj partial ----------------------------
            with tc.tile_pool(name="ps3", bufs=2, space="PSUM") as ps3:
                for st in range(NST):
                    ssl = slice(128 * st, 128 * (st + 1))
                    for dc in range(2):
                        dsl = slice(512 * dc, 512 * (dc + 1))
                        po = ps3.tile([128, 512], F32, name=f"po_{st}_{dc}", tag="po")
                        for pr in range(2):
                            nc.tensor.matmul(
                                po[:], ao_sb[:, pr, ssl], wo_sb[:, pr, dsl],
                                start=(pr == 0), stop=(pr == 1))
                        so = opool.tile([128, 512], F32, name=f"so_{st}_{dc}",
                                        tag="so")
                        nc.scalar.copy(so[:], po[:])
                        nc.sync.dma_start(out_d[ssl, dsl], so[:])

    nc.compile()
    return nc


def _host_inputs(x, Wq, Wk, Wv, Wo, token_positions):
    """Build the 8 per-core input maps (all host-side numpy prep)."""
    x = np.asarray(x, dtype=np.float32)
    Wq = np.asarray(Wq, dtype=np.float32)
    Wk = np.asarray(Wk, dtype=np.float32)
    Wv = np.asarray(Wv, dtype=np.float32)
    Wo = np.asarray(Wo, dtype=np.float32)
    pos = np.asarray(token_positions, dtype=np.int64)

    # RoPE tables per batch: row h*32+j -> cos/sin(pos[s] * freq[j])
    j = np.arange(0, DK, 2, dtype=np.float64) / DK
    freq = 1.0 / (THETA ** j)                       # [32]
    ang = pos[:, None, :] * freq[None, :, None]     # [B, 32, S]
    cos_b = np.tile(np.cos(ang), (1, 4, 1)).astype(np.float32)  # [B, 128, S]
    sin_b = np.tile(np.sin(ang), (1, 4, 1)).astype(np.float32)

    # causal mask patterns for the 4 diagonal offsets
    kk = np.arange(128)[:, None]
    qq = np.arange(SC)[None, :]
    mask_np = np.stack(
        [np.where(qq < kk + 128 * m, NEG, 0.0) for m in range(4)]
    ).astype(np.float32)
    eye_np = np.eye(128, dtype=np.float32)
    ones_np = np.ones((128, NST, HC), dtype=np.float32)

    # RoPE-friendly permutation of Wq/Wk rows within each core's slice:
    # e' = parity*128 + h*32 + j  <-  head h, component 2j+parity
    perm = np.empty(E, dtype=np.int64)
    for p in range(2):
        for h in range(HC):
            for jj in range(32):
                perm[p * 128 + h * 32 + jj] = h * DK + 2 * jj + p

    in_maps = []
    for core in range(8):
        b, g = core // 4, core % 4
        rows = slice(E * g, E * (g + 1))
        wq_c = Wq[rows][perm] * (1.0 / np.sqrt(DK))
        wk_c = Wk[rows][perm]
        in_maps.append({
            "xT": np.ascontiguousarray(x[b].T),
            "wqT": np.ascontiguousarray(wq_c.T.astype(np.float32)),
            "wkT": np.ascontiguousarray(wk_c.T.astype(np.float32)),
            "wvT": np.ascontiguousarray(Wv[rows].T),
            "woT": np.ascontiguousarray(Wo[:, rows].T),
            "cosT": cos_b[b],
            "sinT": sin_b[b],
            "masks": mask_np,
            "eye": eye_np,
            "ones": ones_np,
        })
    return in_maps


def _run(in_maps, trace=False, trace_kwargs=None):
    global _COMPILED
    if _COMPILED is None:
        _COMPILED = _build()
    return run_bass_kernel_spmd(
        _COMPILED, in_maps, list(range(8)), trace=trace,
        **(trace_kwargs or {}))


def _gather(results):
    out = np.empty((B, S, D), dtype=np.float32)
    for b in range(B):
        acc = results[4 * b]["out"].astype(np.float32).copy()
        for g in range(1, 4):
            acc += results[4 * b + g]["out"]
        out[b] = acc
    return out


def kernel(x, Wq, Wk, Wv, Wo, token_positions):
    res = _run(_host_inputs(x, Wq, Wk, Wv, Wo, token_positions))
    return _gather(res.results)


def bench(x, Wq, Wk, Wv, Wo, token_positions):
    """Like kernel() but profiles on HW; returns (out, exec_time_ns)."""
    import types

    try:  # register the NTFF hook if the image's antenv lacks it
        from antenv import axon_hooks  # noqa: F401
    except ImportError:
        m = types.ModuleType("antenv.axon_hooks")
        from trn_agent_boot.trn_boot import _ntff_profile_via_ctypes
        hook = _ntff_profile_via_ctypes("/opt/axon/libaxon_pjrt.so")
        m.get_axon_ntff_profile_hook = lambda: hook
        m.set_axon_ntff_profile_hook = lambda h: None
        sys.modules["antenv.axon_hooks"] = m
        import antenv
        antenv.axon_hooks = m

    res = _run(_host_inputs(x, Wq, Wk, Wv, Wo, token_positions), trace=True)
    return _gather(res.results), res.exec_time_ns


# revision 8
# speedup vs baseline: 1.2938x; 1.2938x over previous
"""Multi-head causal attention with RoPE on 8 Trainium2 NeuronCores.

Problem: x[2, 2048, 1024], 16 heads, d_k=64, RoPE(theta=1e4), causal,
weights W{q,k,v,o}[1024, 1024] stored [d_out, d_in].

Sharding: 2 batches x 4 head-groups -> 8 cores. Core c handles batch
c//4, heads 4*(c%4) .. 4*(c%4)+4. Each core computes its 4 heads'
attention plus the partial o_proj for its head columns; the host sums
the 4 partials per batch (the "all-reduce after o_proj").

Device kernel layout choices (per core):
- Q/K are produced in a permuted [e', s] layout, e' = parity*128 +
  h*32 + j (parity = RoPE pair element, j = rotation freq index), so
  RoPE is 6 full-width DVE ops per 512-seq chunk and the score matmuls
  contract head h over partition rows [32h, 32h+32) of both parity
  tiles (row-packed via tile_position, 4 heads concurrently).
- Scores are computed key-major (scoresT [k, q]) so the attn@V matmul
  needs no transpose and the softmax denominator rides the V matmul as
  an appended ones-column (out row 64).
- The causal mask on diagonal k-tiles is added in PSUM by one extra
  accumulating matmul: identity.T @ mask_pattern (patterns host-built).
- All matmul operands are float32r (TF32-class, full PE rate at N>=256).
"""

import sys

if "/opt/trn_rl_repo" not in sys.path:
    sys.path.insert(0, "/opt/trn_rl_repo")

import numpy as np

import concourse.bass as bass
import concourse.mybir as mybir
import concourse.tile as tile
from concourse import bacc, library_config
from concourse.bass_utils import run_bass_kernel_spmd

F32 = mybir.dt.float32
F32R = mybir.dt.float32r
EXP = mybir.ActivationFunctionType.Exp

B = 2
S = 2048
D = 1024
H = 16
DK = 64
HC = 4          # heads per core
E = HC * DK     # 256 d_out columns per core
THETA = 10000.0
SC = 512        # seq chunk (psum free dim)
NSC = S // SC   # 4
NST = S // 128  # 16 s-tiles
NEG = -1.0e30

_COMPILED = None


def _build():
    nc = bacc.Bacc("TRN2", target_bir_lowering=False, debug=False, num_devices=8)

    xT = nc.dram_tensor("xT", [D, S], F32, kind="ExternalInput")
    wqT = nc.dram_tensor("wqT", [D, E], F32, kind="ExternalInput")
    wkT = nc.dram_tensor("wkT", [D, E], F32, kind="ExternalInput")
    wvT = nc.dram_tensor("wvT", [D, E], F32, kind="ExternalInput")
    woT = nc.dram_tensor("woT", [E, D], F32, kind="ExternalInput")
    cosT = nc.dram_tensor("cosT", [128, S], F32, kind="ExternalInput")
    sinT = nc.dram_tensor("sinT", [128, S], F32, kind="ExternalInput")
    masks = nc.dram_tensor("masks", [4, 128, SC], F32, kind="ExternalInput")
    eye = nc.dram_tensor("eye", [128, 128], F32, kind="ExternalInput")
    ones = nc.dram_tensor("ones", [128, NST, HC], F32, kind="ExternalInput")
    out_d = nc.dram_tensor("out", [S, D], F32, kind="ExternalOutput")

    with tile.TileContext(nc) as tc:
        with (
            tc.tile_pool(name="const", bufs=1) as const,
            tc.tile_pool(name="persist", bufs=1) as persist,
            tc.tile_pool(name="xp", bufs=2) as xp,
            tc.tile_pool(name="ropet", bufs=2) as ropet,
            tc.tile_pool(name="expool", bufs=2) as expool,
            tc.tile_pool(name="rpool", bufs=2) as rpool,
            tc.tile_pool(name="opool", bufs=6) as opool,
            tc.tile_pool(name="cspool", bufs=2) as cspool,
        ):
            nc.gpsimd.load_library(library_config.attn)

            # ---- constant loads -------------------------------------
            wq_sb = const.tile([128, 8, E], F32R)
            wk_sb = const.tile([128, 8, E], F32R)
            wv_sb = const.tile([128, 8, E], F32R)
            nc.sync.dma_start(
                wq_sb[:], wqT[:].rearrange("(c p) e -> p c e", p=128).bitcast(F32R))
            nc.sync.dma_start(
                wk_sb[:], wkT[:].rearrange("(c p) e -> p c e", p=128).bitcast(F32R))
            nc.sync.dma_start(
                wv_sb[:], wvT[:].rearrange("(c p) e -> p c e", p=128).bitcast(F32R))
            wo_sb = const.tile([128, 2, D], F32R)
            nc.sync.dma_start(
                wo_sb[:], woT[:].rearrange("(c p) d -> p c d", p=128).bitcast(F32R))
            mask_sb = const.tile([128, 4, SC], F32R)
            nc.sync.dma_start(
                mask_sb[:], masks[:].rearrange("m k q -> k m q").bitcast(F32R))
            eye_sb = const.tile([128, 128], F32R)
            nc.sync.dma_start(eye_sb[:], eye[:].bitcast(F32R))

            # ---- persistent activations -----------------------------
            q0_sb = persist.tile([128, S], F32R)   # parity-0 rotated Q
            q1_sb = persist.tile([128, S], F32R)
            k0_sb = persist.tile([128, S], F32R)
            k1_sb = persist.tile([128, S], F32R)
            v_sb = persist.tile([128, NST, HC * 65], F32R)  # [k, s_tile, h*65+dk | ones]
            ao_sb = persist.tile([128, 2, S], F32R)         # o_proj lhsT, pair-major

            v3 = v_sb[:].rearrange("p t (h c) -> p t h c", c=65)
            nc.sync.dma_start(
                v3[:, :, :, 64:65],
                ones[:].rearrange("p t h -> p t h 1" if False else "p t (h o) -> p t h o", o=1)
                .bitcast(F32R))

            # ---- stage 1: QKV projections + RoPE + V layout ---------
            with tc.tile_pool(name="ps1", bufs=1, space="PSUM") as ps1:
                for c in range(NSC):
                    sl = slice(SC * c, SC * (c + 1))
                    x_sb = xp.tile([128, 8, SC], F32R, name=f"x_{c}", tag="x")
                    nc.sync.dma_start(
                        x_sb[:],
                        xT[:].rearrange("(dc p) s -> p dc s", p=128)[:, :, sl]
                        .bitcast(F32R))

                    pq = [ps1.tile([128, SC], F32, name=f"pq{t}_{c}", tag=f"pq{t}")
                          for t in range(2)]
                    pk = [ps1.tile([128, SC], F32, name=f"pk{t}_{c}", tag=f"pk{t}")
                          for t in range(2)]
                    for t in range(2):
                        es = slice(128 * t, 128 * (t + 1))
                        for dc in range(8):
                            nc.tensor.matmul(
                                pq[t][:], wq_sb[:, dc, es], x_sb[:, dc, :],
                                start=(dc == 0), stop=(dc == 7))
                        for dc in range(8):
                            nc.tensor.matmul(
                                pk[t][:], wk_sb[:, dc, es], x_sb[:, dc, :],
                                start=(dc == 0), stop=(dc == 7))
                    pv = [ps1.tile([128, 2, 256], F32, name=f"pv{t}_{c}", tag=f"pv{t}")
                          for t in range(2)]
                    for st in range(4):
                        ssl = slice(128 * st, 128 * (st + 1))
                        for dc in range(8):
                            nc.tensor.matmul(
                                pv[st // 2][:, st % 2, :],
                                x_sb[:, dc, ssl], wv_sb[:, dc, :],
                                start=(dc == 0), stop=(dc == 7))

                    # RoPE: x1' = x1 c - x2 s ; x2' = x1 s + x2 c
                    cs_sb = cspool.tile([128, SC], F32, name=f"cos_{c}", tag="cos")
                    sn_sb = cspool.tile([128, SC], F32, name=f"sin_{c}", tag="sin")
                    nc.sync.dma_start(cs_sb[:], cosT[:, sl])
                    nc.sync.dma_start(sn_sb[:], sinT[:, sl])
                    C = cs_sb[:]
                    Sn = sn_sb[:]
                    for name, p0, p1, d0, d1 in (
                        ("q", pq[0], pq[1], q0_sb, q1_sb),
                        ("k", pk[0], pk[1], k0_sb, k1_sb),
                    ):
                        t0 = ropet.tile([128, SC], F32, name=f"t0{name}{c}", tag="ta")
                        t1 = ropet.tile([128, SC], F32, name=f"t1{name}{c}", tag="tb")
                        t2 = ropet.tile([128, SC], F32, name=f"t2{name}{c}", tag="ta")
                        t3 = ropet.tile([128, SC], F32, name=f"t3{name}{c}", tag="tb")
                        nc.vector.tensor_mul(t0[:], p0[:], C)
                        nc.vector.tensor_mul(t1[:], p1[:], Sn)
                        nc.vector.tensor_sub(d0[:, sl], t0[:], t1[:])
                        nc.vector.tensor_mul(t2[:], p0[:], Sn)
                        nc.vector.tensor_mul(t3[:], p1[:], C)
                        nc.vector.tensor_add(d1[:, sl], t2[:], t3[:])

                    # V into [k, h*65+dk] layout (ones col preset above)
                    for st in range(4):
                        nc.scalar.copy(
                            v3[:, 4 * c + st, :, 0:64],
                            pv[st // 2][:, st % 2, :]
                            .rearrange("p (h c) -> p h c", c=64))

            # ---- stage 2: attention ---------------------------------
            with tc.tile_pool(name="ps2", bufs=1, space="PSUM") as ps2:
                for qc in range(NSC):
                    qsl = slice(SC * qc, SC * (qc + 1))
                    av = [ps2.tile([128, SC], F32, name=f"av{h}_{qc}", tag=f"av{h}")
                          for h in range(HC)]
                    nkt = 4 * qc + 4

                    def emit_av(group):
                        for h, ex, kt_, w_ in group:
                            nc.tensor.matmul(
                                av[h][0:65, w_:SC],
                                v_sb[:, kt_, 65 * h:65 * h + 65],
                                ex[:, w_:SC],
                                start=(kt_ == 0), stop=(kt_ == nkt - 1))

                    # software pipeline: scores(kt) | exp(kt) | attnV(kt-1)
                    # so the in-order PE queue never waits on ACT.
                    prev = None
                    for kt in range(nkt):
                        ksl = slice(128 * kt, 128 * (kt + 1))
                        diag = kt >= 4 * qc
                        w = 128 * (kt - 4 * qc) if diag else 0
                        m = kt - 4 * qc
                        cur = []
                        for h in range(HC):
                            hp = slice(32 * h, 32 * (h + 1))
                            tp = (96, 0) if h == 3 else None
                            sc_ps = ps2.tile([128, SC], F32,
                                             name=f"sc{h}_{qc}_{kt}", tag=f"sc{h}")
                            nc.tensor.matmul(
                                sc_ps[:, w:SC], k0_sb[hp, ksl],
                                q0_sb[hp, qsl][:, w:SC],
                                start=True, stop=False, tile_position=tp)
                            nc.tensor.matmul(
                                sc_ps[:, w:SC], k1_sb[hp, ksl],
                                q1_sb[hp, qsl][:, w:SC],
                                start=False, stop=not diag, tile_position=tp)
                            if diag:
                                nc.tensor.matmul(
                                    sc_ps[:, w:SC], eye_sb[:],
                                    mask_sb[:, m, w:SC],
                                    start=False, stop=True)
                            ex = expool.tile([128, SC], F32R,
                                             name=f"ex{h}_{qc}_{kt}", tag=f"ex{h}")
                            cur.append((h, ex, kt, w))
                            nc.scalar.activation(ex[:, w:SC], sc_ps[:, w:SC], EXP)
                        if prev is not None:
                            emit_av(prev)
                        prev = cur
                    emit_av(prev)

                    for h in range(HC):
                        rb = rpool.tile([64, SC], F32, name=f"rb{h}_{qc}", tag="rb")
                        nc.vector.reciprocal(rb[0:1, :], av[h][64:65, :])
                        nc.gpsimd.partition_broadcast(rb[:], rb[0:1, :])
                        u, pr = h % 2, h // 2
                        nc.vector.tensor_mul(
                            ao_sb[64 * u:64 * u + 64, pr, qsl],
                            av[h][0:64, :], rb[:])

            # ---- stage 3: o_proj partial ----------------------------
            with tc.tile_pool(name="ps3", bufs=6, space="PSUM") as ps3:
                for st in range(NST):
                    ssl = slice(128 * st, 128 * (st + 1))
                    for dc in range(2):
                        dsl = slice(512 * dc, 512 * (dc + 1))
                        po = ps3.tile([128, 512], F32, name=f"po_{st}_{dc}", tag="po")
                        for pr in range(2):
                            nc.tensor.matmul(
                                po[:], ao_sb[:, pr, ssl], wo_sb[:, pr, dsl],
                                start=(pr == 0), stop=(pr == 1))
                        so = opool.tile([128, 512], F32, name=f"so_{st}_{dc}",
                                        tag="so")
                        if dc == 0:
                            nc.scalar.copy(so[:], po[:])
                        else:
                            nc.vector.tensor_copy(so[:], po[:])
                        nc.sync.dma_start(out_d[ssl, dsl], so[:])

    nc.compile()
    return nc


def _host_inputs(x, Wq, Wk, Wv, Wo, token_positions):
    """Build the 8 per-core input maps (all host-side numpy prep)."""
    x = np.asarray(x, dtype=np.float32)
    Wq = np.asarray(Wq, dtype=np.float32)
    Wk = np.asarray(Wk, dtype=np.float32)
    Wv = np.asarray(Wv, dtype=np.float32)
    Wo = np.asarray(Wo, dtype=np.float32)
    pos = np.asarray(token_positions, dtype=np.int64)

    # RoPE tables per batch: row h*32+j -> cos/sin(pos[s] * freq[j])
    j = np.arange(0, DK, 2, dtype=np.float64) / DK
    freq = 1.0 / (THETA ** j)                       # [32]
    ang = pos[:, None, :] * freq[None, :, None]     # [B, 32, S]
    cos_b = np.tile(np.cos(ang), (1, 4, 1)).astype(np.float32)  # [B, 128, S]
    sin_b = np.tile(np.sin(ang), (1, 4, 1)).astype(np.float32)

    # causal mask patterns for the 4 diagonal offsets
    kk = np.arange(128)[:, None]
    qq = np.arange(SC)[None, :]
    mask_np = np.stack(
        [np.where(qq < kk + 128 * m, NEG, 0.0) for m in range(4)]
    ).astype(np.float32)
    eye_np = np.eye(128, dtype=np.float32)
    ones_np = np.ones((128, NST, HC), dtype=np.float32)

    # RoPE-friendly permutation of Wq/Wk rows within each core's slice:
    # e' = parity*128 + h*32 + j  <-  head h, component 2j+parity
    perm = np.empty(E, dtype=np.int64)
    for p in range(2):
        for h in range(HC):
            for jj in range(32):
                perm[p * 128 + h * 32 + jj] = h * DK + 2 * jj + p

    in_maps = []
    for core in range(8):
        b, g = core // 4, core % 4
        rows = slice(E * g, E * (g + 1))
        wq_c = Wq[rows][perm] * (1.0 / np.sqrt(DK))
        wk_c = Wk[rows][perm]
        in_maps.append({
            "xT": np.ascontiguousarray(x[b].T),
            "wqT": np.ascontiguousarray(wq_c.T.astype(np.float32)),
            "wkT": np.ascontiguousarray(wk_c.T.astype(np.float32)),
            "wvT": np.ascontiguousarray(Wv[rows].T),
            "woT": np.ascontiguousarray(Wo[:, rows].T),
            "cosT": cos_b[b],
            "sinT": sin_b[b],
            "masks": mask_np,
            "eye": eye_np,
            "ones": ones_np,
        })
    return in_maps


def _run(in_maps, trace=False, trace_kwargs=None):
    global _COMPILED
    if _COMPILED is None:
        _COMPILED = _build()
    return run_bass_kernel_spmd(
        _COMPILED, in_maps, list(range(8)), trace=trace,
        **(trace_kwargs or {}))


def _gather(results):
    out = np.empty((B, S, D), dtype=np.float32)
    for b in range(B):
        acc = results[4 * b]["out"].astype(np.float32).copy()
        for g in range(1, 4):
            acc += results[4 * b + g]["out"]
        out[b] = acc
    return out


def kernel(x, Wq, Wk, Wv, Wo, token_positions):
    res = _run(_host_inputs(x, Wq, Wk, Wv, Wo, token_positions))
    return _gather(res.results)


def bench(x, Wq, Wk, Wv, Wo, token_positions):
    """Like kernel() but profiles on HW; returns (out, exec_time_ns)."""
    import types

    try:  # register the NTFF hook if the image's antenv lacks it
        from antenv import axon_hooks  # noqa: F401
    except ImportError:
        m = types.ModuleType("antenv.axon_hooks")
        from trn_agent_boot.trn_boot import _ntff_profile_via_ctypes
        hook = _ntff_profile_via_ctypes("/opt/axon/libaxon_pjrt.so")
        m.get_axon_ntff_profile_hook = lambda: hook
        m.set_axon_ntff_profile_hook = lambda h: None
        sys.modules["antenv.axon_hooks"] = m
        import antenv
        antenv.axon_hooks = m

    res = _run(_host_inputs(x, Wq, Wk, Wv, Wo, token_positions), trace=True)
    return _gather(res.results), res.exec_time_ns
